# revision 1
# baseline (speedup 1.0000x reference)
"""GQA cross-attention block on 8 trn2 NeuronCores.

Sharding: tensor-parallel over heads. Core c owns KV group g=c (64 dims of
K/V) and its 4 query heads (256 q channels). Each core computes its heads'
attention plus its slice of the o-projection (rows c*256:(c+1)*256 of Wo),
producing a full-shape partial output; the host sums the 8 partials and
adds bo. No device collectives needed.

Device layouts (host prepares):
  xT, encT: [B, HIDDEN, S] bf16 (activations transposed so hidden lands on
  the PE contraction/partition dim), per-core weight slices in bf16,
  biases as [P, 1] fp32 columns for ACT's per-partition bias.

On-chip dataflow per (batch):
  qT [256c, S] = Wq_c^T @ xT   (PE, accum over 16 hidden chunks)
  kT [64, S], vT [64, S] from encT; vT transposed to v_aug [128k, 65]
  chunks with a ones column (row 64 of the AV matmul then yields the
  softmax denominator Z for free).
  scores^T [128k, 512q] = kT-chunk.T @ qT-head-slice (PE)
  E = exp(0.125 * scores) (ACT, PSUM->SBUF bf16)
  av_aug [65, 512q] += v_aug.T @ E (PE, accum over 16 k chunks)
  oT [64, 512q] = av * broadcast(1/Z)  (recip on DVE, broadcast via a
  K=1 PE matmul with a ones row, multiply on DVE)
  out_partial [128s, 512h] = oT.T @ Wo_c (PE) -> bf16 -> DRAM
"""

import numpy as np
import ml_dtypes

import concourse.bass as bass
from concourse import bacc
import concourse.mybir as mybir
import concourse.tile as tile
from concourse.bass_utils import run_bass_kernel_spmd
from concourse.masks import make_identity

BF16 = ml_dtypes.bfloat16
F32 = mybir.dt.float32
BF = mybir.dt.bfloat16

B = 2
S = 2048
HID = 2048
D = 64          # head dim
RQ = 4          # query heads per core (per kv group)
CH = RQ * D     # 256 q channels per core
NCORES = 8
NH = HID // 128  # 16 hidden chunks
NST = S // 512   # 4 s-tiles of 512
NKC = S // 128   # 16 key chunks of 128
SCALE = 1.0 / np.sqrt(D)


def _build_nc() -> bass.Bass:
    nc = bacc.Bacc()

    xT = nc.dram_tensor("xT", [B, HID, S], BF, kind="ExternalInput")
    encT = nc.dram_tensor("encT", [B, HID, S], BF, kind="ExternalInput")
    wq = nc.dram_tensor("wq", [HID, CH], BF, kind="ExternalInput")
    wk = nc.dram_tensor("wk", [HID, D], BF, kind="ExternalInput")
    wv = nc.dram_tensor("wv", [HID, D], BF, kind="ExternalInput")
    wo = nc.dram_tensor("wo", [CH, HID], BF, kind="ExternalInput")
    bq = nc.dram_tensor("bq", [CH, 1], F32, kind="ExternalInput")
    bk = nc.dram_tensor("bk", [D, 1], F32, kind="ExternalInput")
    bv = nc.dram_tensor("bv", [D, 1], F32, kind="ExternalInput")
    out = nc.dram_tensor("out", [B, S, HID], BF, kind="ExternalOutput")

    with tile.TileContext(nc) as tc:
        with (
            tc.tile_pool(name="wpool", bufs=1) as wpool,
            tc.tile_pool(name="xs", bufs=6) as xs_pool,
            tc.tile_pool(name="es", bufs=6) as es_pool,
            tc.tile_pool(name="acts", bufs=2) as acts,
            tc.tile_pool(name="vaug", bufs=2 * NKC) as vaug_pool,
            tc.tile_pool(name="epool", bufs=8) as epool,
            tc.tile_pool(name="small", bufs=4) as small,
            tc.tile_pool(name="osb", bufs=4) as osb_pool,
            tc.tile_pool(name="psum", bufs=2, space="PSUM") as ps,
        ):
            # ---- resident constants / weights ----
            wq_t = []
            wk_t = []
            wv_t = []
            for h in range(NH):
                wqh = wpool.tile([128, CH], BF, name=f"wq{h}")
                nc.sync.dma_start(out=wqh[:], in_=wq[h * 128:(h + 1) * 128, :])
                wq_t.append(wqh)
                wkh = wpool.tile([128, D], BF, name=f"wk{h}")
                nc.sync.dma_start(out=wkh[:], in_=wk[h * 128:(h + 1) * 128, :])
                wk_t.append(wkh)
                wvh = wpool.tile([128, D], BF, name=f"wv{h}")
                nc.sync.dma_start(out=wvh[:], in_=wv[h * 128:(h + 1) * 128, :])
                wv_t.append(wvh)
            wo_t = []
            for cchunk in range(2):
                woc = wpool.tile([128, HID], BF, name=f"wo{cchunk}")
                nc.sync.dma_start(out=woc[:], in_=wo[cchunk * 128:(cchunk + 1) * 128, :])
                wo_t.append(woc)
            bq_t = []
            for cchunk in range(2):
                bqc = wpool.tile([128, 1], F32, name=f"bq{cchunk}")
                nc.sync.dma_start(out=bqc[:], in_=bq[cchunk * 128:(cchunk + 1) * 128, :])
                bq_t.append(bqc)
            bk_t = wpool.tile([D, 1], F32, name="bk_t")
            nc.sync.dma_start(out=bk_t[:], in_=bk[:, :])
            bv_t = wpool.tile([D, 1], F32, name="bv_t")
            nc.sync.dma_start(out=bv_t[:], in_=bv[:, :])

            ident = wpool.tile([128, 128], BF, name="ident")
            make_identity(nc, ident[:])
            ones1 = wpool.tile([1, D], F32, name="ones1")
            nc.gpsimd.memset(ones1[:], 1.0)

            ID = mybir.ActivationFunctionType.Identity
            EXP = mybir.ActivationFunctionType.Exp

            for b in range(B):
                # ---- phase A: projections ----
                qT_h = [
                    acts.tile([D, S], BF, tag=f"q{r}", name=f"q{r}_{b}")
                    for r in range(RQ)
                ]
                kT = acts.tile([D, S], BF, tag="kT", name=f"kT{b}")
                vT = acts.tile([D, S], BF, tag="vT", name=f"vT{b}")

                for st in range(NST):
                    ssl = slice(st * 512, (st + 1) * 512)
                    qps_lo = ps.tile([128, 512], F32, tag="pproj", name=f"qpl{b}{st}")
                    qps_hi = ps.tile([128, 512], F32, tag="pproj", name=f"qph{b}{st}")
                    for h in range(NH):
                        xt = xs_pool.tile([128, 512], BF, tag="xs", name=f"xs{b}{st}{h}")
                        nc.gpsimd.dma_start(
                            out=xt[:], in_=xT[b, h * 128:(h + 1) * 128, ssl])
                        nc.tensor.matmul(
                            qps_lo[:], wq_t[h][:, 0:128], xt[:],
                            start=(h == 0), stop=(h == NH - 1))
                        nc.tensor.matmul(
                            qps_hi[:], wq_t[h][:, 128:256], xt[:],
                            start=(h == 0), stop=(h == NH - 1))
                    for r in range(RQ):
                        src = qps_lo if r < 2 else qps_hi
                        row = (r % 2) * D
                        nc.scalar.activation(
                            qT_h[r][:, ssl], src[row:row + D, :], ID,
                            bias=bq_t[r // 2][row:row + D, :])

                for st in range(NST):
                    ssl = slice(st * 512, (st + 1) * 512)
                    kps = ps.tile([D, 512], F32, tag="pproj", name=f"kps{b}{st}")
                    vps = ps.tile([D, 512], F32, tag="pproj", name=f"vps{b}{st}")
                    for h in range(NH):
                        et = es_pool.tile([128, 512], BF, tag="es", name=f"es{b}{st}{h}")
                        nc.gpsimd.dma_start(
                            out=et[:], in_=encT[b, h * 128:(h + 1) * 128, ssl])
                        nc.tensor.matmul(
                            kps[:], wk_t[h][:], et[:],
                            start=(h == 0), stop=(h == NH - 1))
                        nc.tensor.matmul(
                            vps[:], wv_t[h][:], et[:],
                            start=(h == 0), stop=(h == NH - 1))
                    nc.scalar.activation(kT[:, ssl], kps[:], ID, bias=bk_t[:])
                    nc.scalar.activation(vT[:, ssl], vps[:], ID, bias=bv_t[:])

                # v_aug chunks: [128 kpos, 65] with ones in col 64
                v_aug = []
                for kc in range(NKC):
                    vtp = ps.tile([128, D], BF, tag="ps", name=f"vtp{b}{kc}")
                    nc.tensor.transpose(
                        vtp[:], vT[:, kc * 128:(kc + 1) * 128], ident[0:D, 0:D])
                    va = vaug_pool.tile([128, D + 1], BF, tag="vaug", name=f"va{b}{kc}")
                    nc.gpsimd.memset(va[:, D:D + 1], 1.0)
                    nc.vector.tensor_copy(va[:, 0:D], vtp[:])
                    v_aug.append(va)

                # ---- attention + oT ----
                oT_lo = acts.tile([128, S], BF, tag="olo", name=f"olo{b}")
                oT_hi = acts.tile([128, S], BF, tag="ohi", name=f"ohi{b}")
                for r in range(RQ):
                    odst = oT_lo if r < 2 else oT_hi
                    row = (r % 2) * D
                    avs = [
                        ps.tile([D + 1, 512], F32, tag="pav", bufs=4,
                                name=f"av{b}{r}{qc}")
                        for qc in range(NST)
                    ]
                    # kc-outer, qc pairs inner: stationary (kT chunk /
                    # v_aug chunk) reused across consecutive matmuls, and
                    # all four av accumulators stay hot in PSUM.
                    for kc in range(NKC):
                        ksl = slice(kc * 128, (kc + 1) * 128)
                        for pair in range(NST // 2):
                            scs = []
                            for qc in (2 * pair, 2 * pair + 1):
                                qsl = slice(qc * 512, (qc + 1) * 512)
                                sct = ps.tile([128, 512], F32, tag="ps",
                                              name=f"sc{b}{r}{qc}{kc}")
                                nc.tensor.matmul(
                                    sct[:], kT[:, ksl], qT_h[r][:, qsl],
                                    start=True, stop=True)
                                e_t = epool.tile([128, 512], BF, tag="e",
                                                 name=f"e{b}{r}{qc}{kc}")
                                nc.scalar.activation(
                                    e_t[:], sct[:], EXP, scale=float(SCALE))
                                scs.append(e_t)
                            for j, qc in enumerate((2 * pair, 2 * pair + 1)):
                                nc.tensor.matmul(
                                    avs[qc][:], v_aug[kc][:], scs[j][:],
                                    start=(kc == 0), stop=(kc == NKC - 1))
                    for qc in range(NST):
                        qsl = slice(qc * 512, (qc + 1) * 512)
                        av = avs[qc]
                        rt = small.tile([1, 512], F32, tag="rt", name=f"rt{b}{r}{qc}")
                        nc.vector.reciprocal(rt[:], av[D:D + 1, :])
                        bc = ps.tile([D, 512], F32, tag="ps", name=f"bc{b}{r}{qc}")
                        nc.tensor.matmul(bc[:], ones1[:], rt[:], start=True, stop=True)
                        bcs = small.tile([D, 512], F32, tag="bcs", name=f"bcs{b}{r}{qc}")
                        nc.vector.tensor_copy(bcs[:], bc[:])
                        nc.vector.tensor_mul(odst[row:row + D, qsl], av[0:D, :], bcs[:])

                # ---- o-projection (partial over this core's 256 channels) ----
                for sc16 in range(S // 128):
                    s128 = slice(sc16 * 128, (sc16 + 1) * 128)
                    for hc in range(HID // 512):
                        hsl = slice(hc * 512, (hc + 1) * 512)
                        ops = ps.tile([128, 512], F32, tag="pproj", name=f"op{b}{sc16}{hc}")
                        nc.tensor.matmul(
                            ops[:], oT_lo[:, s128], wo_t[0][:, hsl],
                            start=True, stop=False)
                        nc.tensor.matmul(
                            ops[:], oT_hi[:, s128], wo_t[1][:, hsl],
                            start=False, stop=True)
                        osb = osb_pool.tile([128, 512], BF, tag="osb", name=f"ob{b}{sc16}{hc}")
                        nc.vector.tensor_copy(osb[:], ops[:])
                        nc.sync.dma_start(out=out[b, s128, hsl], in_=osb[:])

    if not nc.is_finalized():
        nc.finalize()
    return nc


_NC = None
_RUN_KWARGS = {}
_LAST_RESULT = None


def _get_nc():
    global _NC
    if _NC is None:
        _NC = _build_nc()
    return _NC


def kernel(x, encoder_output, Wq, bq, Wk, bk, Wv, bv, Wo, bo):
    nc = _get_nc()
    xT = np.ascontiguousarray(np.asarray(x, np.float32).transpose(0, 2, 1)).astype(BF16)
    encT = np.ascontiguousarray(
        np.asarray(encoder_output, np.float32).transpose(0, 2, 1)).astype(BF16)
    Wq = np.asarray(Wq, np.float32)
    Wk = np.asarray(Wk, np.float32)
    Wv = np.asarray(Wv, np.float32)
    Wo = np.asarray(Wo, np.float32)
    in_maps = []
    for c in range(NCORES):
        csl = slice(c * CH, (c + 1) * CH)
        gsl = slice(c * D, (c + 1) * D)
        in_maps.append({
            "xT": xT,
            "encT": encT,
            "wq": np.ascontiguousarray(Wq[:, csl]).astype(BF16),
            "wk": np.ascontiguousarray(Wk[:, gsl]).astype(BF16),
            "wv": np.ascontiguousarray(Wv[:, gsl]).astype(BF16),
            "wo": np.ascontiguousarray(Wo[csl, :]).astype(BF16),
            "bq": np.ascontiguousarray(
                np.asarray(bq, np.float32)[csl].reshape(CH, 1)),
            "bk": np.ascontiguousarray(
                np.asarray(bk, np.float32)[gsl].reshape(D, 1)),
            "bv": np.ascontiguousarray(
                np.asarray(bv, np.float32)[gsl].reshape(D, 1)),
        })
    res = run_bass_kernel_spmd(nc, in_maps, list(range(NCORES)), **_RUN_KWARGS)
    global _LAST_RESULT
    _LAST_RESULT = res
    total = np.zeros((B, S, HID), np.float32)
    for c in range(NCORES):
        total += res.results[c]["out"].astype(np.float32)
    return total + np.asarray(bo, np.float32)



# revision 7
# speedup vs baseline: 1.2377x; 1.2377x over previous
"""GQA cross-attention block on 8 trn2 NeuronCores.

Sharding: tensor-parallel over heads. Core c owns KV group g=c (64 dims of
K/V) and its 4 query heads (256 q channels). Each core computes its heads'
attention plus its slice of the o-projection (rows c*256:(c+1)*256 of Wo),
producing a full-shape partial output; the host sums the 8 partials and
adds bo. No device collectives needed.

Device layouts (host prepares):
  xT, encT: [B, HIDDEN, S] bf16, per-core weight slices in bf16 (K and V
  projection weights packed as one [HID, 128] stationary), biases as
  [P, 1] fp32 columns.

Structure per batch:
  projections (PE) -> v_aug build (PE transposes) -> attention in
  (head, qc-pair) units: scores pipelined one k-chunk ahead of AV so the
  exp (ACT) latency never stalls the PE, softmax epilogue of the previous
  unit (DVE reciprocal -> fp16 ones-matmul broadcast -> DVE multiply)
  deferred under the current unit's matmuls -> o-projection.
"""

import numpy as np
import ml_dtypes

import concourse.bass as bass
from concourse import bacc
import concourse.mybir as mybir
import concourse.tile as tile
from concourse.bass_utils import run_bass_kernel_spmd
from concourse.masks import make_identity

BF16 = ml_dtypes.bfloat16
F32 = mybir.dt.float32
F16 = mybir.dt.float16
BF = mybir.dt.bfloat16

B = 2
S = 2048
HID = 2048
D = 64          # head dim
RQ = 4          # query heads per core (per kv group)
CH = RQ * D     # 256 q channels per core
NCORES = 8
NH = HID // 128  # 16 hidden chunks
NST = S // 512   # 4 s-tiles of 512
NKC = S // 128   # 16 key chunks of 128
SCALE = 1.0 / np.sqrt(D)


def _build_nc() -> bass.Bass:
    nc = bacc.Bacc()

    xT = nc.dram_tensor("xT", [B, HID, S], BF, kind="ExternalInput")
    encT = nc.dram_tensor("encT", [B, HID, S], BF, kind="ExternalInput")
    wq = nc.dram_tensor("wq", [HID, CH], BF, kind="ExternalInput")
    wkv = nc.dram_tensor("wkv", [HID, 2 * D], BF, kind="ExternalInput")
    wo = nc.dram_tensor("wo", [CH, HID], BF, kind="ExternalInput")
    bq = nc.dram_tensor("bq", [CH, 1], F32, kind="ExternalInput")
    bkv = nc.dram_tensor("bkv", [2 * D, 1], F32, kind="ExternalInput")
    out = nc.dram_tensor("out", [B, S, HID], BF, kind="ExternalOutput")

    with tile.TileContext(nc) as tc:
        with (
            tc.tile_pool(name="wpool", bufs=1) as wpool,
            tc.tile_pool(name="xs", bufs=6) as xs_pool,
            tc.tile_pool(name="es", bufs=6) as es_pool,
            tc.tile_pool(name="acts", bufs=2) as acts,
            tc.tile_pool(name="vaug", bufs=2 * NKC) as vaug_pool,
            tc.tile_pool(name="epool", bufs=6) as epool,
            tc.tile_pool(name="small", bufs=4) as small,
            tc.tile_pool(name="osb", bufs=4) as osb_pool,
            tc.tile_pool(name="psum", bufs=3, space="PSUM") as ps,
        ):
            # ---- resident weights ----
            wq_t = []
            wkv_t = []
            for h in range(NH):
                wqh = wpool.tile([128, CH], BF, name=f"wq{h}")
                nc.sync.dma_start(out=wqh[:], in_=wq[h * 128:(h + 1) * 128, :])
                wq_t.append(wqh)
                wkvh = wpool.tile([128, 2 * D], BF, name=f"wkv{h}")
                nc.sync.dma_start(out=wkvh[:], in_=wkv[h * 128:(h + 1) * 128, :])
                wkv_t.append(wkvh)
            wo_t = []
            for cchunk in range(2):
                woc = wpool.tile([128, HID], BF, name=f"wo{cchunk}")
                nc.sync.dma_start(out=woc[:], in_=wo[cchunk * 128:(cchunk + 1) * 128, :])
                wo_t.append(woc)
            bq_t = []
            for cchunk in range(2):
                bqc = wpool.tile([128, 1], F32, name=f"bq{cchunk}")
                nc.sync.dma_start(out=bqc[:], in_=bq[cchunk * 128:(cchunk + 1) * 128, :])
                bq_t.append(bqc)
            bkv_t = wpool.tile([2 * D, 1], F32, name="bkv_t")
            nc.sync.dma_start(out=bkv_t[:], in_=bkv[:, :])

            ident = wpool.tile([128, 128], BF, name="ident")
            make_identity(nc, ident[:])
            ones1 = wpool.tile([1, D], F16, name="ones1")
            nc.gpsimd.memset(ones1[:], 1.0)

            ID = mybir.ActivationFunctionType.Identity
            EXP = mybir.ActivationFunctionType.Exp

            # persistent v_aug tiles; ones column written once
            va_tiles = [
                [vaug_pool.tile([128, D + 1], BF, tag="vaug", name=f"va{b}_{kc}")
                 for kc in range(NKC)]
                for b in range(B)
            ]
            for b in range(B):
                for kc in range(NKC):
                    nc.gpsimd.memset(va_tiles[b][kc][:, D:D + 1], 1.0)

            # deferred softmax epilogues: (av_tile, r, qc, odst, b)
            pending = []

            def flush_epilogue():
                while pending:
                    av, r, qc, odst, b = pending.pop(0)
                    qsl = slice(qc * 512, (qc + 1) * 512)
                    row = (r % 2) * D
                    rt = small.tile([1, 512], F16, tag="rt", name=f"rt{b}{r}{qc}")
                    with nc.allow_low_precision(reason="1/Z in fp16 is ample"):
                        nc.vector.reciprocal(rt[:], av[D:D + 1, :])
                    bcp = ps.tile([D, 512], F32, tag="bc", bufs=1,
                                  name=f"bcp{b}{r}{qc}")
                    nc.tensor.matmul(bcp[:], ones1[:], rt[:], start=True, stop=True)
                    bcs = small.tile([D, 512], F32, tag="bcs", name=f"bcs{b}{r}{qc}")
                    nc.vector.tensor_copy(bcs[:], bcp[:])
                    nc.vector.tensor_mul(odst[row:row + D, qsl], av[0:D, :], bcs[:])

            for b in range(B):
                # ---- projections ----
                qT_h = [
                    acts.tile([D, S], BF, tag=f"q{r}", name=f"q{r}_{b}")
                    for r in range(RQ)
                ]
                kT = acts.tile([D, S], BF, tag="kT", name=f"kT{b}")
                vT = acts.tile([D, S], BF, tag="vT", name=f"vT{b}")

                for st in range(NST):
                    ssl = slice(st * 512, (st + 1) * 512)
                    qps_lo = ps.tile([128, 512], F32, tag="big", name=f"qpl{b}{st}")
                    qps_hi = ps.tile([128, 512], F32, tag="big", name=f"qph{b}{st}")
                    kvps = ps.tile([128, 512], F32, tag="big", name=f"kvps{b}{st}")
                    for h in range(NH):
                        xt = xs_pool.tile([128, 512], BF, tag="xs", name=f"xs{b}{st}{h}")
                        nc.gpsimd.dma_start(
                            out=xt[:], in_=xT[b, h * 128:(h + 1) * 128, ssl])
                        et = es_pool.tile([128, 512], BF, tag="es", name=f"es{b}{st}{h}")
                        nc.sync.dma_start(
                            out=et[:], in_=encT[b, h * 128:(h + 1) * 128, ssl])
                        nc.tensor.matmul(
                            qps_lo[:], wq_t[h][:, 0:128], xt[:],
                            start=(h == 0), stop=(h == NH - 1))
                        nc.tensor.matmul(
                            qps_hi[:], wq_t[h][:, 128:256], xt[:],
                            start=(h == 0), stop=(h == NH - 1))
                        nc.tensor.matmul(
                            kvps[:], wkv_t[h][:], et[:],
                            start=(h == 0), stop=(h == NH - 1))
                    for r in range(RQ):
                        src = qps_lo if r < 2 else qps_hi
                        row = (r % 2) * D
                        nc.scalar.activation(
                            qT_h[r][:, ssl], src[row:row + D, :], ID,
                            bias=bq_t[r // 2][row:row + D, :])
                    nc.scalar.activation(
                        kT[:, ssl], kvps[0:D, :], ID, bias=bkv_t[0:D, :])
                    nc.scalar.activation(
                        vT[:, ssl], kvps[D:2 * D, :], ID, bias=bkv_t[D:2 * D, :])
                    # v_aug chunks for this s-tile
                    for j in range(4):
                        kc = st * 4 + j
                        vtp = ps.tile([128, D], BF, tag="big", name=f"vtp{b}{kc}")
                        nc.tensor.transpose(
                            vtp[:], vT[:, kc * 128:(kc + 1) * 128], ident[0:D, 0:D])
                        nc.vector.tensor_copy(va_tiles[b][kc][:, 0:D], vtp[:])

                # ---- attention: units of (head, qc pair) ----
                oT_lo = acts.tile([128, S], BF, tag="olo", name=f"olo{b}")
                oT_hi = acts.tile([128, S], BF, tag="ohi", name=f"ohi{b}")
                for r in range(RQ):
                    odst = oT_lo if r < 2 else oT_hi
                    for pair in range(2):
                        qa, qb = 2 * pair, 2 * pair + 1
                        sla = slice(qa * 512, (qa + 1) * 512)
                        slb = slice(qb * 512, (qb + 1) * 512)
                        ava = ps.tile([D + 1, 512], F32, tag="pav", bufs=4,
                                      name=f"av{b}{r}{qa}")
                        avb = ps.tile([D + 1, 512], F32, tag="pav", bufs=4,
                                      name=f"av{b}{r}{qb}")
                        es = [None] * NKC
                        for kc in range(NKC):
                            ksl = slice(kc * 128, (kc + 1) * 128)
                            for qc, qsl in ((qa, sla), (qb, slb)):
                                sct = ps.tile([128, 512], F32, tag="big",
                                              name=f"sc{b}{r}{qc}{kc}")
                                e2 = epool.tile([128, 512], BF, tag="e",
                                                name=f"e{b}{r}{qc}{kc}")
                                nc.tensor.matmul(
                                    sct[:], kT[:, ksl], qT_h[r][:, qsl],
                                    start=True, stop=True)
                                nc.scalar.activation(
                                    e2[:], sct[:], EXP, scale=float(SCALE))
                                if qc == qa:
                                    es[kc] = [e2]
                                else:
                                    es[kc].append(e2)
                            # AV lags one k-chunk so exp latency never
                            # stalls the PE stream.
                            if kc > 0:
                                pea, peb = es[kc - 1]
                                nc.tensor.matmul(
                                    ava[:], va_tiles[b][kc - 1][:], pea[:],
                                    start=(kc == 1), stop=False)
                                nc.tensor.matmul(
                                    avb[:], va_tiles[b][kc - 1][:], peb[:],
                                    start=(kc == 1), stop=False)
                            if kc == 3:
                                flush_epilogue()
                        pea, peb = es[NKC - 1]
                        nc.tensor.matmul(
                            ava[:], va_tiles[b][NKC - 1][:], pea[:],
                            start=False, stop=True)
                        nc.tensor.matmul(
                            avb[:], va_tiles[b][NKC - 1][:], peb[:],
                            start=False, stop=True)
                        pending.append((ava, r, qa, odst, b))
                        pending.append((avb, r, qb, odst, b))

                flush_epilogue()

                # ---- o-projection (partial over this core's 256 channels) ----
                for sc16 in range(S // 128):
                    s128 = slice(sc16 * 128, (sc16 + 1) * 128)
                    for hc in range(4):
                        hsl = slice(hc * 512, (hc + 1) * 512)
                        ops = ps.tile([128, 512], F32, tag="big",
                                      name=f"op{b}{sc16}{hc}")
                        nc.tensor.matmul(
                            ops[:], oT_lo[:, s128], wo_t[0][:, hsl],
                            start=True, stop=False)
                        nc.tensor.matmul(
                            ops[:], oT_hi[:, s128], wo_t[1][:, hsl],
                            start=False, stop=True)
                        osb = osb_pool.tile([128, 512], BF, tag="osb",
                                            name=f"ob{b}{sc16}{hc}")
                        if hc % 2 == 0:
                            nc.vector.tensor_copy(osb[:], ops[:])
                        else:
                            nc.scalar.copy(osb[:], ops[:])
                        nc.sync.dma_start(out=out[b, s128, hsl], in_=osb[:])

    if not nc.is_finalized():
        nc.finalize()
    return nc


_NC = None
_RUN_KWARGS = {}
_LAST_RESULT = None


def _get_nc():
    global _NC
    if _NC is None:
        _NC = _build_nc()
    return _NC


def kernel(x, encoder_output, Wq, bq, Wk, bk, Wv, bv, Wo, bo):
    nc = _get_nc()
    xT = np.ascontiguousarray(np.asarray(x, np.float32).transpose(0, 2, 1)).astype(BF16)
    encT = np.ascontiguousarray(
        np.asarray(encoder_output, np.float32).transpose(0, 2, 1)).astype(BF16)
    Wq = np.asarray(Wq, np.float32)
    Wk = np.asarray(Wk, np.float32)
    Wv = np.asarray(Wv, np.float32)
    Wo = np.asarray(Wo, np.float32)
    bk = np.asarray(bk, np.float32)
    bv = np.asarray(bv, np.float32)
    in_maps = []
    for c in range(NCORES):
        csl = slice(c * CH, (c + 1) * CH)
        gsl = slice(c * D, (c + 1) * D)
        in_maps.append({
            "xT": xT,
            "encT": encT,
            "wq": np.ascontiguousarray(Wq[:, csl]).astype(BF16),
            "wkv": np.ascontiguousarray(
                np.concatenate([Wk[:, gsl], Wv[:, gsl]], axis=1)).astype(BF16),
            "wo": np.ascontiguousarray(Wo[csl, :]).astype(BF16),
            "bq": np.ascontiguousarray(
                np.asarray(bq, np.float32)[csl].reshape(CH, 1)),
            "bkv": np.ascontiguousarray(
                np.concatenate([bk[gsl], bv[gsl]]).reshape(2 * D, 1)),
        })
    res = run_bass_kernel_spmd(nc, in_maps, list(range(NCORES)), **_RUN_KWARGS)
    global _LAST_RESULT
    _LAST_RESULT = res
    total = np.zeros((B, S, HID), np.float32)
    for c in range(NCORES):
        total += res.results[c]["out"].astype(np.float32)
    return total + np.asarray(bo, np.float32)
